# revision 1
# baseline (speedup 1.0000x reference)
"""Trainium2 Bass kernel for the contrastive loss problem.

Sharding: core c handles sentence-loss for secrets [4c, 4c+4) (upper-triangle
tiles of the BxB distance matrices, x2-minus-diagonal trick) and secret-loss
for batch columns [128c, 128c+128). Per-core scalar partials are summed on the
host (equivalent to the all-reduce of the scalar losses).
"""

import sys

sys.path.insert(0, "/opt/trn_rl_repo")

import numpy as np
import ml_dtypes

import concourse.bacc as bacc
import concourse.tile as tile
from concourse import mybir
from concourse.bass_utils import run_bass_kernel_spmd

N, B, D = 32, 1024, 1024
NCORES = 8
SECPC = N // NCORES  # 4 secrets per core (sentence term)
BSH = B // NCORES  # 128 batch columns per core (secret term)
EPS = 1e-12
MARGIN = 1.0
ALPHA = 0.5
RSQRT2 = 0.7071067811865476  # Square(x * 1/sqrt(2)) == x^2 / 2

f32 = mybir.dt.float32
bf16 = mybir.dt.bfloat16
fp16 = mybir.dt.float16
Alu = mybir.AluOpType
Act = mybir.ActivationFunctionType
AxX = mybir.AxisListType.X


def _segs(mi):
    """Column segments (start, width<=512) covering [128*mi, 1024)."""
    out = []
    s = 128 * mi
    while s < B:
        w = min(512, B - s)
        out.append((s, w))
        s += w
    return out


N_SEG = sum(len(_segs(mi)) for mi in range(8))  # 12
DS_OFF = {}  # mi -> packed column offset of DS storage
_o = 0
for _mi in range(8):
    DS_OFF[_mi] = _o
    _o += B - 128 * _mi
DS_W = _o  # 4608


def _build():
    nc = bacc.Bacc("TRN2", target_bir_lowering=False, debug=False, num_devices=NCORES)

    xs_ap = nc.dram_tensor("xs", [SECPC, B, D], f32, kind="ExternalInput").ap()
    xsec_ap = nc.dram_tensor("xsec", [N, BSH, D], f32, kind="ExternalInput").ap()
    enc_ap = nc.dram_tensor("enc", [B, D], f32, kind="ExternalInput").ap()
    idb_ap = nc.dram_tensor("identb", [128, 128], fp16, kind="ExternalInput").ap()
    um_ap = nc.dram_tensor("umask", [32, 512], f32, kind="ExternalInput").ap()
    o_sent_ap = nc.dram_tensor("o_sent", [128, 2], f32, kind="ExternalOutput").ap()
    o_sec_ap = nc.dram_tensor("o_sec", [32, 1], f32, kind="ExternalOutput").ap()

    with tile.TileContext(nc) as tc:
        _body(tc, nc, xs_ap, xsec_ap, enc_ap, idb_ap, um_ap, o_sent_ap, o_sec_ap)
    nc.compile()
    return nc


def _body(tc, nc, xs_ap, xsec_ap, enc_ap, idb_ap, um_ap, o_sent_ap, o_sec_ap):
    import contextlib

    with contextlib.ExitStack() as ctx:
        cpool = ctx.enter_context(tc.tile_pool(name="consts", bufs=1))
        spool = ctx.enter_context(tc.tile_pool(name="slots", bufs=1))
        dram_pool = ctx.enter_context(tc.tile_pool(name="dram", bufs=1, space="DRAM"))

        ident_b = cpool.tile([128, 128], fp16, tag="identb")
        nc.sync.dma_start(ident_b[:], idb_ap[:])
        umask = cpool.tile([32, 512], f32, tag="umask")
        nc.sync.dma_start(umask[:], um_ap[:])
        eps_t = cpool.tile([128, 1], f32, tag="epst")
        nc.vector.memset(eps_t[:], EPS)
        ones128 = cpool.tile([1, 128], fp16, tag="ones128")
        nc.vector.memset(ones128[:], 1.0)
        ones32 = cpool.tile([1, 32], fp16, tag="ones32")
        nc.vector.memset(ones32[:], 1.0)

        sent_slots = spool.tile([128, SECPC * N_SEG], f32, tag="sent_slots")
        accd_slots = spool.tile([128, SECPC * 8], f32, tag="accd_slots")
        sec_slots = spool.tile([32, 8], f32, tag="sec_slots")

        # ---------------- sentence (distance consistency) phase ----------------
        with contextlib.ExitStack() as tctx:
            xnat_pool = tctx.enter_context(tc.tile_pool(name="xnat", bufs=2))
            xtb_pool = tctx.enter_context(tc.tile_pool(name="xtb", bufs=2))
            sq_pool = tctx.enter_context(tc.tile_pool(name="sqp", bufs=2))
            ds_pool = tctx.enter_context(tc.tile_pool(name="dsp", bufs=1))
            junk_pool = tctx.enter_context(tc.tile_pool(name="tjunk", bufs=2))
            ptp_pool = tctx.enter_context(
                tc.tile_pool(name="ptp_t", bufs=4, space="PSUM")
            )
            pmm_pool = tctx.enter_context(
                tc.tile_pool(name="pmm_t", bufs=4, space="PSUM")
            )
            work_pool = tctx.enter_context(tc.tile_pool(name="twork", bufs=3))

            ds = ds_pool.tile([128, DS_W], f32, tag="ds")

            def process_matrix(src3d, is_ds, si_base, di_base):
                """src3d: [p, t, d] AP view (f32 in DRAM). Computes grams over the
                upper-triangle tile region; writes DS if is_ds else accumulates
                (d - ds)^2 into sent_slots/accd_slots."""
                xnat = xnat_pool.tile([128, 8, D], fp16, tag="xnat")
                nc.gpsimd.dma_start(xnat[:], src3d)
                sq2 = sq_pool.tile([128, 8], f32, tag="sq2")
                for t in range(8):
                    junk = junk_pool.tile([128, D], fp16, tag="tjunk")
                    nc.scalar.activation(
                        out=junk[:],
                        in_=xnat[:, t, :],
                        func=Act.Square,
                        scale=RSQRT2,
                        accum_out=sq2[:, t : t + 1],
                    )
                # sqrow[0, t, p] = -0.5*|x_(128t+p)|^2 in row-form on partition 0
                # (rank-1 matmul operand) — bounce through DRAM scratch.
                sqn2 = sq_pool.tile([128, 8], f32, tag="sqn2")
                nc.scalar.activation(out=sqn2[:], in_=sq2[:], func=Act.Copy, scale=-1.0)
                scr = dram_pool.tile([8, 128], f32, tag="scr_sent")
                nc.sync.dma_start(scr[:].rearrange("t p -> p t"), sqn2[:])
                sqrow = sq_pool.tile([1, 8, 128], fp16, tag="sqrow")
                nc.gpsimd.dma_start(sqrow[:], scr[:][None])

                xtb = xtb_pool.tile([128, 8, B], fp16, tag="xtb")
                for k in range(8):
                    for t in range(8):
                        pst = ptp_pool.tile([128, 128], fp16, tag="pstt")
                        nc.tensor.transpose(
                            pst[:], xnat[:, t, 128 * k : 128 * (k + 1)], ident_b[:]
                        )
                        nc.vector.tensor_copy(
                            xtb[:, k, 128 * t : 128 * (t + 1)], pst[:]
                        )

                si = si_base
                di = di_base
                for mi in range(8):
                    for (s, w) in _segs(mi):
                        ps = pmm_pool.tile([128, 512], f32, tag="ps_mm")
                        for k in range(8):
                            nc.tensor.matmul(
                                ps[:, :w],
                                xtb[:, k, 128 * mi : 128 * (mi + 1)],
                                xtb[:, k, s : s + w],
                                start=(k == 0),
                                stop=False,
                            )
                        # rank-1 updates: add -0.5*sq_b along free columns
                        tlo = s // 128
                        thi = (s + w - 1) // 128
                        for t in range(tlo, thi + 1):
                            a0 = max(s, 128 * t)
                            a1 = min(s + w, 128 * (t + 1))
                            nc.tensor.matmul(
                                ps[:, a0 - s : a1 - s],
                                ones128[:],
                                sqrow[0:1, t, a0 - 128 * t : a1 - 128 * t],
                                start=False,
                                stop=(t == thi),
                            )
                        # m = min(g - sq_b/2 - sq_a/2, 0) = -relu(d2)/2
                        m = work_pool.tile([128, 512], f32, tag="tmin")
                        nc.vector.tensor_scalar(
                            out=m[:, :w],
                            in0=ps[:, :w],
                            scalar1=sq2[:, mi : mi + 1],
                            scalar2=0.0,
                            op0=Alu.subtract,
                            op1=Alu.min,
                        )
                        off = DS_OFF[mi] + (s - 128 * mi)
                        if is_ds:
                            nc.scalar.activation(
                                out=ds[:, off : off + w],
                                in_=m[:, :w],
                                func=Act.Sqrt,
                                scale=-2.0,
                                bias=eps_t[:],
                            )
                        else:
                            d = work_pool.tile([128, 512], f32, tag="td")
                            nc.scalar.activation(
                                out=d[:, :w],
                                in_=m[:, :w],
                                func=Act.Sqrt,
                                scale=-2.0,
                                bias=eps_t[:],
                            )
                            diff = work_pool.tile([128, 512], f32, tag="tdiff")
                            nc.vector.scalar_tensor_tensor(
                                out=diff[:, :w],
                                in0=d[:, :w],
                                scalar=0.0,
                                in1=ds[:, off : off + w],
                                op0=Alu.bypass,
                                op1=Alu.subtract,
                            )
                            junk2 = work_pool.tile([128, 512], f32, tag="tjunk2")
                            nc.vector.scalar_tensor_tensor(
                                out=junk2[:, :w],
                                in0=diff[:, :w],
                                scalar=0.0,
                                in1=diff[:, :w],
                                op0=Alu.bypass,
                                op1=Alu.mult,
                                accum_out=sent_slots[:, si : si + 1],
                            )
                            si += 1
                            if s == 128 * mi:
                                junk3 = work_pool.tile([128, 128], f32, tag="tjunk3")
                                nc.vector.scalar_tensor_tensor(
                                    out=junk3[:],
                                    in0=diff[:, :128],
                                    scalar=0.0,
                                    in1=diff[:, :128],
                                    op0=Alu.bypass,
                                    op1=Alu.mult,
                                    accum_out=accd_slots[:, di : di + 1],
                                )
                                di += 1

            process_matrix(enc_ap.rearrange("(t p) d -> p t d", p=128), True, 0, 0)
            for i in range(SECPC):
                process_matrix(
                    xs_ap[i].rearrange("(t p) d -> p t d", p=128),
                    False,
                    i * N_SEG,
                    i * 8,
                )

        # ---------------- secret (pairwise margin) phase ----------------
        with contextlib.ExitStack() as sctx:
            xsn_pool = sctx.enter_context(tc.tile_pool(name="xsn", bufs=2))
            xts_pool = sctx.enter_context(tc.tile_pool(name="xtsec", bufs=1))
            sqs_pool = sctx.enter_context(tc.tile_pool(name="sqsec", bufs=1))
            junk_pool = sctx.enter_context(tc.tile_pool(name="sjunk", bufs=2))
            ptp_pool = sctx.enter_context(
                tc.tile_pool(name="ptp_s", bufs=3, space="PSUM")
            )
            pmm_pool = sctx.enter_context(
                tc.tile_pool(name="pmm_s", bufs=2, space="PSUM")
            )
            work_pool = sctx.enter_context(tc.tile_pool(name="swork", bufs=3))

            # xtsec[d, k, i, b] = outputs[i, 128c + b, 128k + d]
            xtsec = xts_pool.tile([128, 8, N, BSH], fp16, tag="xtsec")
            sqsec2 = sqs_pool.tile([128, N], f32, tag="sqsec2")  # 0.5*|x|^2
            for g in range(4):
                xsn = xsn_pool.tile([128, 8, D], fp16, tag="xsn")
                nc.gpsimd.dma_start(
                    xsn[:], xsec_ap[8 * g : 8 * g + 8].rearrange("i b d -> b i d")
                )
                for ii in range(8):
                    i = 8 * g + ii
                    junk = junk_pool.tile([128, D], fp16, tag="sjunk")
                    nc.scalar.activation(
                        out=junk[:],
                        in_=xsn[:, ii, :],
                        func=Act.Square,
                        scale=RSQRT2,
                        accum_out=sqsec2[:, i : i + 1],
                    )
                    for k in range(8):
                        pst = ptp_pool.tile([128, 128], fp16, tag="pst")
                        nc.tensor.transpose(
                            pst[:], xsn[:, ii, 128 * k : 128 * (k + 1)], ident_b[:]
                        )
                        nc.vector.tensor_copy(xtsec[:, k, i, :], pst[:])
            # -0.5*|x|^2 in row-form [1, b, i] on partition 0 (matmul operands
            # must start at partition 0/32/64) — bounce through DRAM scratch.
            sqsecn = sqs_pool.tile([128, N], f32, tag="sqsecn")
            nc.scalar.activation(out=sqsecn[:], in_=sqsec2[:], func=Act.Copy, scale=-1.0)
            scr_sec = dram_pool.tile([BSH, N], f32, tag="scr_sec")
            nc.sync.dma_start(scr_sec[:], sqsecn[:])
            sqsrow = sqs_pool.tile([1, BSH, N], fp16, tag="sqsrow")
            nc.gpsimd.dma_start(sqsrow[:], scr_sec[:][None])

            for g8 in range(8):  # 16 b's per group
                ps = pmm_pool.tile([32, 512], f32, tag="ps_sec")
                for bb in range(16):
                    b = 16 * g8 + bb
                    c0 = 32 * bb
                    for k in range(8):
                        op = xtsec[:, k, :, b]
                        nc.tensor.matmul(
                            ps[:, c0 : c0 + 32], op, op, start=(k == 0), stop=False
                        )
                    nc.tensor.matmul(
                        ps[:, c0 : c0 + 32],
                        sqsrow[0:1, b, :],
                        ones32[:],
                        start=False,
                        stop=False,
                    )
                    nc.tensor.matmul(
                        ps[:, c0 : c0 + 32],
                        ones32[:],
                        sqsrow[0:1, b, :],
                        start=False,
                        stop=True,
                    )
                # ps = g - (sq_i + sq_j)/2 = -d2/2
                m = work_pool.tile([32, 512], f32, tag="smin")
                nc.vector.tensor_scalar(
                    out=m[:], in0=ps[:], scalar1=0.0, scalar2=None, op0=Alu.min
                )
                dse = work_pool.tile([32, 512], f32, tag="sdse")
                nc.scalar.activation(
                    out=dse[:], in_=m[:], func=Act.Sqrt, scale=-2.0, bias=eps_t[0:32]
                )
                hin = work_pool.tile([32, 512], f32, tag="shin")
                nc.scalar.activation(
                    out=hin[:], in_=dse[:], func=Act.Relu, scale=-1.0, bias=float(MARGIN)
                )
                junk2 = work_pool.tile([32, 512], f32, tag="sjunk2")
                nc.vector.scalar_tensor_tensor(
                    out=junk2[:],
                    in0=hin[:],
                    scalar=0.0,
                    in1=umask[:],
                    op0=Alu.bypass,
                    op1=Alu.mult,
                    accum_out=sec_slots[:, g8 : g8 + 1],
                )

        # ---------------- final reduction + output ----------------
        with tc.tile_pool(name="outp", bufs=1) as opool:
            o_sent = opool.tile([128, 2], f32, tag="o_sent_sb")
            nc.vector.tensor_reduce(
                out=o_sent[:, 0:1], in_=sent_slots[:], axis=AxX, op=Alu.add
            )
            nc.vector.tensor_reduce(
                out=o_sent[:, 1:2], in_=accd_slots[:], axis=AxX, op=Alu.add
            )
            nc.sync.dma_start(o_sent_ap[:], o_sent[:])
            o_sec = opool.tile([32, 1], f32, tag="o_sec_sb")
            nc.vector.tensor_reduce(
                out=o_sec[:], in_=sec_slots[:], axis=AxX, op=Alu.add
            )
            nc.sync.dma_start(o_sec_ap[:], o_sec[:])


_NC_CACHE = None


def _get_nc():
    global _NC_CACHE
    if _NC_CACHE is None:
        _NC_CACHE = _build()
    return _NC_CACHE


def _host_inputs():
    ident_b = np.eye(128, dtype=np.float16)
    um = np.tile(np.triu(np.ones((32, 32), np.float32), 1), (1, 16))
    return ident_b, um


def run_on_device(outputs, encode_sentences, trace=False, **kw):
    nc = _get_nc()
    ident_b, um = _host_inputs()
    in_maps = []
    for c in range(NCORES):
        in_maps.append(
            {
                "xs": np.ascontiguousarray(outputs[SECPC * c : SECPC * (c + 1)]),
                "xsec": np.ascontiguousarray(outputs[:, BSH * c : BSH * (c + 1), :]),
                "enc": np.ascontiguousarray(encode_sentences),
                "identb": ident_b,
                "umask": um,
            }
        )
    return run_bass_kernel_spmd(nc, in_maps, list(range(NCORES)), trace=trace, **kw)


def _finish(results):
    sent_region = 0.0
    diag = 0.0
    sec = 0.0
    for c in range(NCORES):
        r = results[c]
        sent_region += r["o_sent"][:, 0].sum(dtype=np.float64)
        diag += r["o_sent"][:, 1].sum(dtype=np.float64)
        sec += r["o_sec"].sum(dtype=np.float64)
    total_sent = 2.0 * sent_region - diag
    sentence_loss = total_sent / (N * B * B)
    secret_loss = (sec / B) / (N * (N - 1) / 2.0)
    loss = ALPHA * sentence_loss + (1.0 - ALPHA) * secret_loss
    return (
        np.float32(loss),
        np.float32(sentence_loss),
        np.float32(secret_loss),
    )


def kernel(outputs, encode_sentences):
    res = run_on_device(outputs, encode_sentences)
    return _finish(res.results)



# revision 2
# speedup vs baseline: 1.0961x; 1.0961x over previous
"""Trainium2 Bass kernel for the contrastive loss problem (v2).

Sharding: core c handles sentence-loss for secrets [4c, 4c+4) (upper-triangle
block-row tiles of the BxB distance matrices, x2-minus-diagonal trick) and
secret-loss for batch columns [128c, 128c+128). Per-core scalar partials are
summed on the host (equivalent to the all-reduce of the scalar losses).

v2 redesign vs baseline:
 - secret-phase grams packed 4 batch items per 128-wide matmul (4x fewer PE
   instructions; keeps the PE warm instead of HAM-throttled)
 - sq_a folded into the sqrt bias (per-partition AP), single rank-1 per
   segment with f32 ones(-1) stationary, contiguous f32 sqrow
 - transposes batched 8-per-PSUM-bank with one wide copy
 - f32 bounce DMAs ride the HWDGE queue, not the SWDGE cast queue
"""

import sys

sys.path.insert(0, "/opt/trn_rl_repo")

import numpy as np

import concourse.bacc as bacc
import concourse.tile as tile
from concourse import mybir
from concourse.bass_utils import run_bass_kernel_spmd

N, B, D = 32, 1024, 1024
NCORES = 8
SECPC = N // NCORES  # 4 secrets per core (sentence term)
BSH = B // NCORES  # 128 batch columns per core (secret term)
NG = BSH // 4  # 32 packed groups of 4 b's (secret term)
EPS = 1e-12
MARGIN = 1.0
ALPHA = 0.5

f32 = mybir.dt.float32
fp16 = mybir.dt.float16
Alu = mybir.AluOpType
Act = mybir.ActivationFunctionType
AxX = mybir.AxisListType.X


def _segs(mi):
    """Column segments (start, width<=512) covering [128*mi, 1024)."""
    out = []
    s = 128 * mi
    while s < B:
        w = min(512, B - s)
        out.append((s, w))
        s += w
    return out


N_SEG = sum(len(_segs(mi)) for mi in range(8))  # 12
DS_OFF = {}  # mi -> packed column offset of DS storage
_o = 0
for _mi in range(8):
    DS_OFF[_mi] = _o
    _o += B - 128 * _mi
DS_W = _o  # 4608


def _build():
    nc = bacc.Bacc("TRN2", target_bir_lowering=False, debug=False, num_devices=NCORES)

    xs_ap = nc.dram_tensor("xs", [SECPC, B, D], f32, kind="ExternalInput").ap()
    xsp_ap = nc.dram_tensor("xsecp", [NG, 128, D], f32, kind="ExternalInput").ap()
    enc_ap = nc.dram_tensor("enc", [B, D], f32, kind="ExternalInput").ap()
    idb_ap = nc.dram_tensor("identb", [128, 128], fp16, kind="ExternalInput").ap()
    pm_ap = nc.dram_tensor("pmask", [128, 128], fp16, kind="ExternalInput").ap()
    o_sent_ap = nc.dram_tensor("o_sent", [128, 2], f32, kind="ExternalOutput").ap()
    o_sec_ap = nc.dram_tensor("o_sec", [128, 1], f32, kind="ExternalOutput").ap()

    with tile.TileContext(nc) as tc:
        _body(tc, nc, xs_ap, xsp_ap, enc_ap, idb_ap, pm_ap, o_sent_ap, o_sec_ap)
    nc.compile()
    return nc


def _body(tc, nc, xs_ap, xsp_ap, enc_ap, idb_ap, pm_ap, o_sent_ap, o_sec_ap):
    import contextlib

    with contextlib.ExitStack() as ctx:
        cpool = ctx.enter_context(tc.tile_pool(name="consts", bufs=1))
        spool = ctx.enter_context(tc.tile_pool(name="slots", bufs=1))
        dram_pool = ctx.enter_context(tc.tile_pool(name="dram", bufs=2, space="DRAM"))

        ident_b = cpool.tile([128, 128], fp16, tag="identb")
        nc.sync.dma_start(ident_b[:], idb_ap[:])
        pmask = cpool.tile([128, 128], fp16, tag="pmask")
        nc.sync.dma_start(pmask[:], pm_ap[:])
        ones_neg = cpool.tile([1, 128], f32, tag="onesneg")
        nc.vector.memset(ones_neg[:], -1.0)

        sent_slots = spool.tile([128, SECPC * N_SEG], f32, tag="sent_slots")
        accd_slots = spool.tile([128, SECPC * 8], f32, tag="accd_slots")
        sec_slots = spool.tile([128, NG], f32, tag="sec_slots")

        # shared PSUM pools (<=8 banks total)
        tp_pool = ctx.enter_context(tc.tile_pool(name="tp_ps", bufs=3, space="PSUM"))
        mm_pool = ctx.enter_context(tc.tile_pool(name="mm_ps", bufs=4, space="PSUM"))

        # ---------------- sentence (distance consistency) phase ----------------
        with contextlib.ExitStack() as tctx:
            xnat_pool = tctx.enter_context(tc.tile_pool(name="xnat", bufs=2))
            xtb_pool = tctx.enter_context(tc.tile_pool(name="xtb", bufs=2))
            sq_pool = tctx.enter_context(tc.tile_pool(name="sqp", bufs=2))
            ds_pool = tctx.enter_context(tc.tile_pool(name="dsp", bufs=1))
            work_pool = tctx.enter_context(tc.tile_pool(name="twork", bufs=4))

            ds = ds_pool.tile([128, DS_W], fp16, tag="ds")

            def process_matrix(src3d, is_ds, si_base, di_base):
                """src3d: [p, t, d] AP view (f32 in DRAM)."""
                # --- load (2 chunks of 4 t's) + squares + bounce ---
                xnat = xnat_pool.tile([128, 8, D], fp16, tag="xnat")
                nc.gpsimd.dma_start(xnat[:, 0:4, :], src3d[:, 0:4, :])
                nc.gpsimd.dma_start(xnat[:, 4:8, :], src3d[:, 4:8, :])
                sq2 = sq_pool.tile([128, 8], f32, tag="sq2")  # 0.5*|x|^2
                junk16 = None
                for t in range(8):
                    junk16 = work_pool.tile([128, D], fp16, tag="junk16")
                    nc.vector.scalar_tensor_tensor(
                        out=junk16[:],
                        in0=xnat[:, t, :],
                        scalar=0.5,
                        in1=xnat[:, t, :],
                        op0=Alu.mult,
                        op1=Alu.mult,
                        accum_out=sq2[:, t : t + 1],
                    )
                sbias = sq_pool.tile([128, 8], f32, tag="sbias")  # |x|^2 + eps
                nc.vector.tensor_scalar(
                    out=sbias[:],
                    in0=sq2[:],
                    scalar1=2.0,
                    scalar2=EPS,
                    op0=Alu.mult,
                    op1=Alu.add,
                )
                # sqrow[0, 128t+p] = 0.5*|x_(128t+p)|^2 on partition 0 (f32,
                # HWDGE bounce so it skips the big SWDGE cast queue)
                scr = dram_pool.tile([8, 128], f32, tag="scr_sent")
                nc.sync.dma_start(scr[:].rearrange("t p -> p t"), sq2[:])
                sqrow = sq_pool.tile([1, 8, 128], f32, tag="sqrow")
                nc.sync.dma_start(sqrow[:], scr[:][None])
                sqrow_f = sqrow[:].rearrange("o t p -> o (t p)")

                # --- transposes: 8 per PSUM bank, 1 wide copy each ---
                xtb = xtb_pool.tile([128, 8, B], fp16, tag="xtb")
                for t in range(8):
                    tp = tp_pool.tile([128, 8, 128], fp16, tag="tp")
                    for k in range(8):
                        nc.tensor.transpose(
                            tp[:, k, :], xnat[:, t, 128 * k : 128 * (k + 1)], ident_b[:]
                        )
                    if t % 2 == 0:
                        nc.vector.tensor_copy(xtb[:, :, 128 * t : 128 * (t + 1)], tp[:])
                    else:
                        nc.scalar.copy(xtb[:, :, 128 * t : 128 * (t + 1)], tp[:])

                # --- grams + postproc ---
                si = si_base
                di = di_base
                for mi in range(8):
                    segs = _segs(mi)
                    pss = []
                    for (s, w) in segs:
                        ps = mm_pool.tile([128, 512], f32, tag="ps_mm")
                        pss.append(ps)
                        # rank-1: add -0.5*|x_b|^2 along free dim
                        nc.tensor.matmul(
                            ps[:, :w],
                            ones_neg[:],
                            sqrow_f[0:1, s : s + w],
                            start=True,
                            stop=False,
                        )
                    for k in range(8):
                        for (s, w), ps in zip(segs, pss):
                            nc.tensor.matmul(
                                ps[:, :w],
                                xtb[:, k, 128 * mi : 128 * (mi + 1)],
                                xtb[:, k, s : s + w],
                                start=False,
                                stop=(k == 7),
                            )
                    for (s, w), ps in zip(segs, pss):
                        off = DS_OFF[mi] + (s - 128 * mi)
                        is_diag = s == 128 * mi
                        if is_diag:
                            # clamp so -2*ps + |x_a|^2 + eps >= 0 on the
                            # a==b diagonal (sqrt of rounding noise)
                            mcl = work_pool.tile([128, 128], f32, tag="mcl")
                            nc.vector.tensor_scalar(
                                out=mcl[:],
                                in0=ps[:, 0:128],
                                scalar1=sq2[:, mi : mi + 1],
                                scalar2=None,
                                op0=Alu.min,
                            )
                            d0 = ds[:, off : off + 128] if is_ds else None
                            if not is_ds:
                                dtile = work_pool.tile([128, 512], fp16, tag="dtile")
                                d0 = dtile[:, 0:128]
                            nc.scalar.activation(
                                out=d0,
                                in_=mcl[:],
                                func=Act.Sqrt,
                                scale=-2.0,
                                bias=sbias[:, mi : mi + 1],
                            )
                            if w > 128:
                                d1 = (
                                    ds[:, off + 128 : off + w]
                                    if is_ds
                                    else dtile[:, 128:w]
                                )
                                nc.scalar.activation(
                                    out=d1,
                                    in_=ps[:, 128:w],
                                    func=Act.Sqrt,
                                    scale=-2.0,
                                    bias=sbias[:, mi : mi + 1],
                                )
                        else:
                            if is_ds:
                                dtarget = ds[:, off : off + w]
                            else:
                                dtile = work_pool.tile([128, 512], fp16, tag="dtile")
                                dtarget = dtile[:, :w]
                            nc.scalar.activation(
                                out=dtarget,
                                in_=ps[:, :w],
                                func=Act.Sqrt,
                                scale=-2.0,
                                bias=sbias[:, mi : mi + 1],
                            )
                        if not is_ds:
                            diff = work_pool.tile([128, 512], fp16, tag="tdiff")
                            nc.vector.scalar_tensor_tensor(
                                out=diff[:, :w],
                                in0=dtile[:, :w],
                                scalar=0.0,
                                in1=ds[:, off : off + w],
                                op0=Alu.bypass,
                                op1=Alu.subtract,
                            )
                            junk = work_pool.tile([128, 512], fp16, tag="tjunk")
                            nc.scalar.activation(
                                out=junk[:, :w],
                                in_=diff[:, :w],
                                func=Act.Square,
                                accum_out=sent_slots[:, si : si + 1],
                            )
                            si += 1
                            if is_diag:
                                junk2 = work_pool.tile([128, 128], fp16, tag="tjunk2")
                                nc.vector.scalar_tensor_tensor(
                                    out=junk2[:],
                                    in0=diff[:, 0:128],
                                    scalar=0.0,
                                    in1=diff[:, 0:128],
                                    op0=Alu.bypass,
                                    op1=Alu.mult,
                                    accum_out=accd_slots[:, di : di + 1],
                                )
                                di += 1

            process_matrix(enc_ap.rearrange("(t p) d -> p t d", p=128), True, 0, 0)
            for i in range(SECPC):
                process_matrix(
                    xs_ap[i].rearrange("(t p) d -> p t d", p=128),
                    False,
                    i * N_SEG,
                    i * 8,
                )

        # ---------------- secret (pairwise margin) phase ----------------
        # packed layout: partition p = 32*bb + i  (bb = b within group of 4)
        with contextlib.ExitStack() as sctx:
            xsn_pool = sctx.enter_context(tc.tile_pool(name="xsn", bufs=2))
            tg_pool = sctx.enter_context(tc.tile_pool(name="tg", bufs=3))
            sqs_pool = sctx.enter_context(tc.tile_pool(name="sqsec", bufs=2))
            swork_pool = sctx.enter_context(tc.tile_pool(name="swork", bufs=4))

            for l in range(NG // 8):  # 4 loads of 8 groups
                xsn = xsn_pool.tile([128, 8, D], fp16, tag="xsn")
                nc.gpsimd.dma_start(
                    xsn[:], xsp_ap[8 * l : 8 * l + 8].rearrange("g p d -> p g d")
                )
                sq2s = sqs_pool.tile([128, 8], f32, tag="sq2s")  # 0.5*|x|^2
                for gg in range(8):
                    junk16 = swork_pool.tile([128, D], fp16, tag="sjunk16")
                    nc.vector.scalar_tensor_tensor(
                        out=junk16[:],
                        in0=xsn[:, gg, :],
                        scalar=0.5,
                        in1=xsn[:, gg, :],
                        op0=Alu.mult,
                        op1=Alu.mult,
                        accum_out=sq2s[:, gg : gg + 1],
                    )
                sbias_s = sqs_pool.tile([128, 8], f32, tag="sbias_s")
                nc.vector.tensor_scalar(
                    out=sbias_s[:],
                    in0=sq2s[:],
                    scalar1=2.0,
                    scalar2=EPS,
                    op0=Alu.mult,
                    op1=Alu.add,
                )
                scr_s = dram_pool.tile([8, 128], f32, tag="scr_sec")
                nc.sync.dma_start(scr_s[:].rearrange("g p -> p g"), sq2s[:])
                sqsrow = sqs_pool.tile([1, 8, 128], f32, tag="sqsrow")
                nc.sync.dma_start(sqsrow[:], scr_s[:][None])

                for half in range(2):  # supergroup = 4 groups -> 1 gram bank
                    psb = mm_pool.tile([128, 512], f32, tag="ps_mm")
                    tgs = []
                    for q in range(4):
                        gg = 4 * half + q
                        g = 8 * l + gg
                        # transposes -> T_g[d, k, (bb,i)]
                        tp = tp_pool.tile([128, 8, 128], fp16, tag="tp")
                        for k in range(8):
                            nc.tensor.transpose(
                                tp[:, k, :],
                                xsn[:, gg, 128 * k : 128 * (k + 1)],
                                ident_b[:],
                            )
                        tg = tg_pool.tile([128, 8, 128], fp16, tag="tg")
                        tgs.append(tg)
                        if q % 2 == 0:
                            nc.vector.tensor_copy(tg[:], tp[:])
                        else:
                            nc.scalar.copy(tg[:], tp[:])
                        # rank-1 (-0.5|x_j|^2 along free) + grams
                        nc.tensor.matmul(
                            psb[:, 128 * q : 128 * (q + 1)],
                            ones_neg[:],
                            sqsrow[0:1, gg, :],
                            start=True,
                            stop=False,
                        )
                        for k in range(8):
                            nc.tensor.matmul(
                                psb[:, 128 * q : 128 * (q + 1)],
                                tg[:, k, :],
                                tg[:, k, :],
                                start=False,
                                stop=(k == 7),
                            )
                    for q in range(4):
                        gg = 4 * half + q
                        g = 8 * l + gg
                        msec = swork_pool.tile([128, 128], f32, tag="msec")
                        nc.vector.tensor_scalar(
                            out=msec[:],
                            in0=psb[:, 128 * q : 128 * (q + 1)],
                            scalar1=sq2s[:, gg : gg + 1],
                            scalar2=None,
                            op0=Alu.min,
                        )
                        dsec = swork_pool.tile([128, 128], fp16, tag="dsec")
                        nc.scalar.activation(
                            out=dsec[:],
                            in_=msec[:],
                            func=Act.Sqrt,
                            scale=-2.0,
                            bias=sbias_s[:, gg : gg + 1],
                        )
                        hin = swork_pool.tile([128, 128], fp16, tag="shin")
                        nc.scalar.activation(
                            out=hin[:],
                            in_=dsec[:],
                            func=Act.Relu,
                            scale=-1.0,
                            bias=float(MARGIN),
                        )
                        junk3 = swork_pool.tile([128, 128], fp16, tag="sjunk3")
                        nc.vector.scalar_tensor_tensor(
                            out=junk3[:],
                            in0=hin[:],
                            scalar=0.0,
                            in1=pmask[:],
                            op0=Alu.bypass,
                            op1=Alu.mult,
                            accum_out=sec_slots[:, g : g + 1],
                        )

        # ---------------- final reduction + output ----------------
        with tc.tile_pool(name="outp", bufs=1) as opool:
            o_sent = opool.tile([128, 2], f32, tag="o_sent_sb")
            nc.vector.tensor_reduce(
                out=o_sent[:, 0:1], in_=sent_slots[:], axis=AxX, op=Alu.add
            )
            nc.vector.tensor_reduce(
                out=o_sent[:, 1:2], in_=accd_slots[:], axis=AxX, op=Alu.add
            )
            nc.sync.dma_start(o_sent_ap[:], o_sent[:])
            o_sec = opool.tile([128, 1], f32, tag="o_sec_sb")
            nc.vector.tensor_reduce(
                out=o_sec[:], in_=sec_slots[:], axis=AxX, op=Alu.add
            )
            nc.sync.dma_start(o_sec_ap[:], o_sec[:])


_NC_CACHE = None


def _get_nc():
    global _NC_CACHE
    if _NC_CACHE is None:
        _NC_CACHE = _build()
    return _NC_CACHE


def _host_inputs():
    ident_b = np.eye(128, dtype=np.float16)
    # pmask[(bb,i),(bb2,j)] = 1 if bb==bb2 and i<j else 0
    bb = np.arange(128) // 32
    ii = np.arange(128) % 32
    pm = ((bb[:, None] == bb[None, :]) & (ii[:, None] < ii[None, :])).astype(
        np.float16
    )
    return ident_b, pm


def run_on_device(outputs, encode_sentences, trace=False, **kw):
    nc = _get_nc()
    ident_b, pm = _host_inputs()
    in_maps = []
    for c in range(NCORES):
        xsl = outputs[:, BSH * c : BSH * (c + 1), :]  # [32, 128, D]
        # packed: [g, 32*bb+i, d] with b = 128c + 4g + bb
        xsp = np.ascontiguousarray(xsl.transpose(1, 0, 2)).reshape(NG, 4 * N, D)
        in_maps.append(
            {
                "xs": np.ascontiguousarray(outputs[SECPC * c : SECPC * (c + 1)]),
                "xsecp": xsp,
                "enc": np.ascontiguousarray(encode_sentences),
                "identb": ident_b,
                "pmask": pm,
            }
        )
    return run_bass_kernel_spmd(nc, in_maps, list(range(NCORES)), trace=trace, **kw)


def _finish(results):
    sent_region = 0.0
    diag = 0.0
    sec = 0.0
    for c in range(NCORES):
        r = results[c]
        sent_region += r["o_sent"][:, 0].sum(dtype=np.float64)
        diag += r["o_sent"][:, 1].sum(dtype=np.float64)
        sec += r["o_sec"].sum(dtype=np.float64)
    total_sent = 2.0 * sent_region - diag
    sentence_loss = total_sent / (N * B * B)
    secret_loss = (sec / B) / (N * (N - 1) / 2.0)
    loss = ALPHA * sentence_loss + (1.0 - ALPHA) * secret_loss
    return (
        np.float32(loss),
        np.float32(sentence_loss),
        np.float32(secret_loss),
    )


def kernel(outputs, encode_sentences):
    res = run_on_device(outputs, encode_sentences)
    return _finish(res.results)


# revision 9
# speedup vs baseline: 1.6408x; 1.4969x over previous
"""Trainium2 Bass kernel for the contrastive loss problem (v4).

Sharding: core c handles sentence-loss for secrets [4c, 4c+4) (upper-triangle
block-row tiles of the BxB distance matrices, x2-minus-diagonal trick) and
secret-loss for batch columns [128c, 128c+128). Per-core scalar partials are
summed on the host (equivalent to the all-reduce of the scalar losses).

v5 design:
 - |x|^2 extracted from the raw gram diagonal via a full-bank DVE evacuation
   copy + identity-mask STT (PSUM banks keep a one-reader/one-writer
   discipline: concurrent PE-write + ACT/DVE-read of one bank corrupts)
 - the 4 rank-1s of a packed bank are ONE 512-wide matmul, so every
   quarter's postproc RAW-depends on the bank's last PE write
 - sq row vector built on-chip (8 single-column PE transposes), no DRAM
   bounce on the critical path
 - sq_a folded into the sqrt bias = 2*(S - fp16(S/2)) + eps (diag exactly 0)
 - secret-phase grams packed 4 b's per 128-wide matmul; batches interleaved
   between sentence matrices (DMA pacing + PE density)
"""

import sys

sys.path.insert(0, "/opt/trn_rl_repo")

import numpy as np

import concourse.bacc as bacc
import concourse.tile as tile
from concourse import mybir
from concourse.bass_utils import run_bass_kernel_spmd

N, B, D = 32, 1024, 1024
NCORES = 8
SECPC = N // NCORES  # 4 secrets per core (sentence term)
BSH = B // NCORES  # 128 batch columns per core (secret term)
NG = BSH // 4  # 32 packed groups of 4 b's (secret term)
EPS = 1e-12
MARGIN = 1.0
ALPHA = 0.5

f32 = mybir.dt.float32
fp16 = mybir.dt.float16
Alu = mybir.AluOpType
Act = mybir.ActivationFunctionType
AxX = mybir.AxisListType.X


def _nsegs(mi):
    """Non-diagonal column segments (start, width<=512) covering
    [128*(mi+1), 1024)."""
    out = []
    s = 128 * (mi + 1)
    while s < B:
        w = min(512, B - s)
        out.append((s, w))
        s += w
    return out


N_SEG = sum(len(_nsegs(mi)) for mi in range(8))  # 10 non-diag segs
DS_OFF = {}  # mi -> packed column offset of DS storage (incl diag block)
_o = 0
for _mi in range(8):
    DS_OFF[_mi] = _o
    _o += B - 128 * _mi
DS_W = _o  # 4608


def _build():
    nc = bacc.Bacc("TRN2", target_bir_lowering=False, debug=False, num_devices=NCORES)

    xs_ap = nc.dram_tensor("xs", [SECPC, B, D], f32, kind="ExternalInput").ap()
    xsp_ap = nc.dram_tensor("xsecp", [NG, 128, D], f32, kind="ExternalInput").ap()
    enc_ap = nc.dram_tensor("enc", [B, D], f32, kind="ExternalInput").ap()
    idb_ap = nc.dram_tensor("identb", [128, 128], fp16, kind="ExternalInput").ap()
    pm_ap = nc.dram_tensor("pmask", [128, 128], fp16, kind="ExternalInput").ap()
    o_sent_ap = nc.dram_tensor("o_sent", [128, 2], f32, kind="ExternalOutput").ap()
    o_sec_ap = nc.dram_tensor("o_sec", [128, 1], f32, kind="ExternalOutput").ap()

    with tile.TileContext(nc) as tc:
        _body(tc, nc, xs_ap, xsp_ap, enc_ap, idb_ap, pm_ap, o_sent_ap, o_sec_ap)
    nc.compile()
    return nc


def _body(tc, nc, xs_ap, xsp_ap, enc_ap, idb_ap, pm_ap, o_sent_ap, o_sec_ap):
    import contextlib

    with contextlib.ExitStack() as ctx:
        cpool = ctx.enter_context(tc.tile_pool(name="consts", bufs=1))
        spool = ctx.enter_context(tc.tile_pool(name="slots", bufs=1))

        ident_b = cpool.tile([128, 128], fp16, tag="identb")
        nc.sync.dma_start(ident_b[:], idb_ap[:])
        pmask = cpool.tile([128, 128], fp16, tag="pmask")
        nc.sync.dma_start(pmask[:], pm_ap[:])
        ones_neg = cpool.tile([1, 128], fp16, tag="onesneg")
        nc.vector.memset(ones_neg[:], -1.0)

        sent_slots = spool.tile([128, SECPC * N_SEG], f32, tag="sent_slots")
        accd_slots = spool.tile([128, SECPC * 8], f32, tag="accd_slots")
        sec_slots = spool.tile([128, NG], f32, tag="sec_slots")

        tp_pool = ctx.enter_context(tc.tile_pool(name="tp_ps", bufs=2, space="PSUM"))
        dg_pool = ctx.enter_context(tc.tile_pool(name="dg_ps", bufs=2, space="PSUM"))
        mm_pool = ctx.enter_context(tc.tile_pool(name="mm_ps", bufs=4, space="PSUM"))

        xnat_pool = ctx.enter_context(tc.tile_pool(name="xnat", bufs=3))
        xtb_pool = ctx.enter_context(tc.tile_pool(name="xtb", bufs=2))
        sq_pool = ctx.enter_context(tc.tile_pool(name="sqp", bufs=2))
        ds_pool = ctx.enter_context(tc.tile_pool(name="dsp", bufs=1))
        work_pool = ctx.enter_context(tc.tile_pool(name="twork", bufs=4))
        xsn_pool = ctx.enter_context(tc.tile_pool(name="xsn", bufs=2))
        tg_pool = ctx.enter_context(tc.tile_pool(name="tg", bufs=3))
        sqs_pool = ctx.enter_context(tc.tile_pool(name="sqsec", bufs=2))

        ds = ds_pool.tile([128, DS_W], fp16, tag="ds")

        def sq_chain(dpss, sq_p, nb, tag):
            """Extract S from nb packed diag banks -> (sq2, sbias, sqrow).
            dpss: list of nb psum banks, 4 diag blocks each."""
            # full-bank evacuation (sole PSUM reader before the rank-1)
            raws = []
            for h in range(nb):
                raw = work_pool.tile([128, 512], f32, tag=f"{tag}raw")
                raws.append(raw)
                nc.vector.tensor_copy(raw[:], dpss[h][:])
            sq2 = sq_p.tile([128, 4 * nb], f32, tag=f"{tag}sq2")
            for h in range(nb):
                for q in range(4):
                    j = 4 * h + q
                    junk = work_pool.tile([128, 128], fp16, tag=f"{tag}xjunk")
                    nc.vector.scalar_tensor_tensor(
                        out=junk[:],
                        in0=raws[h][:, 128 * q : 128 * (q + 1)],
                        scalar=0.0,
                        in1=ident_b[:],
                        op0=Alu.bypass,
                        op1=Alu.mult,
                        accum_out=sq2[:, j : j + 1],
                    )
            sq16 = sq_p.tile([128, 4 * nb], fp16, tag=f"{tag}sq16")
            nc.vector.tensor_scalar(
                out=sq16[:], in0=sq2[:], scalar1=0.5, scalar2=None, op0=Alu.mult
            )
            sbias = sq_p.tile([128, 4 * nb], f32, tag=f"{tag}sbias")
            nc.vector.scalar_tensor_tensor(
                out=sbias[:],
                in0=sq2[:],
                scalar=1.0,
                in1=sq16[:],
                op0=Alu.mult,
                op1=Alu.subtract,
            )
            nc.vector.tensor_scalar(
                out=sbias[:],
                in0=sbias[:],
                scalar1=2.0,
                scalar2=EPS,
                op0=Alu.mult,
                op1=Alu.add,
            )
            # row form via single-column PE transposes (no DRAM bounce);
            # rides a "tp"-tagged bank (partition 0 only) to stay in budget
            tprow = tp_pool.tile([128, 8, 128], fp16, tag="tp")
            for j in range(4 * nb):
                nc.tensor.transpose(tprow[0:1, j, :], sq16[:, j : j + 1], ident_b[:])
            sqrow = sq_p.tile([1, 4 * nb, 128], fp16, tag=f"{tag}sqrow")
            nc.vector.tensor_copy(sqrow[:], tprow[0:1, 0 : 4 * nb, :])
            return sq2, sbias, sqrow, raws

        def load_matrix(src3d, nchunk):
            xnat = xnat_pool.tile([128, 8, D], fp16, tag="xnat")
            step = 8 // nchunk
            for c0 in range(0, 8, step):
                nc.gpsimd.dma_start(
                    xnat[:, c0 : c0 + step, :], src3d[:, c0 : c0 + step, :]
                )
            return xnat

        def process_matrix(xnat, is_ds, si_base, di_base):
            # --- transposes: 8 per PSUM bank, 1 wide copy each ---
            xtb = xtb_pool.tile([128, 8, B], fp16, tag="xtb")
            for t in range(8):
                tp = tp_pool.tile([128, 8, 128], fp16, tag="tp")
                for k in range(8):
                    nc.tensor.transpose(
                        tp[:, k, :], xnat[:, t, 128 * k : 128 * (k + 1)], ident_b[:]
                    )
                nc.vector.tensor_copy(xtb[:, :, 128 * t : 128 * (t + 1)], tp[:])

            # --- diag-block grams (raw), packed 4 per bank ---
            dpss = []
            for h in range(2):
                dps = dg_pool.tile([128, 512], f32, tag="dps")
                dpss.append(dps)
                for q in range(4):
                    mi = 4 * h + q
                    blk = slice(128 * mi, 128 * (mi + 1))
                    for k in range(8):
                        nc.tensor.matmul(
                            dps[:, 128 * q : 128 * (q + 1)],
                            xtb[:, k, blk],
                            xtb[:, k, blk],
                            start=(k == 0),
                            stop=False,
                        )
            sq2, sbias, sqrow, raws = sq_chain(dpss, sq_pool, 2, "m")
            sqrow_f = sqrow[:].rearrange("o t p -> o (t p)")

            # --- non-diag grams (single rank-1 last) + inline postproc ---
            si = si_base
            di = di_base
            for mi in range(8):
                for (s, w) in _nsegs(mi):
                    ps = mm_pool.tile([128, 512], f32, tag="ps_mm")
                    blk = slice(128 * mi, 128 * (mi + 1))
                    for k in range(8):
                        nc.tensor.matmul(
                            ps[:, :w],
                            xtb[:, k, blk],
                            xtb[:, k, s : s + w],
                            start=(k == 0),
                            stop=False,
                        )
                    nc.tensor.matmul(
                        ps[:, :w],
                        ones_neg[:],
                        sqrow_f[0:1, s : s + w],
                        start=False,
                        stop=True,
                    )
                    if is_ds:
                        off0 = DS_OFF[mi] + s - 128 * mi
                        dtarget = ds[:, off0 : off0 + w]
                    else:
                        dtile = work_pool.tile([128, 512], fp16, tag="dtile")
                        dtarget = dtile[:, :w]
                    nc.scalar.activation(
                        out=dtarget,
                        in_=ps[:, :w],
                        func=Act.Sqrt,
                        scale=-2.0,
                        bias=sbias[:, mi : mi + 1],
                    )
                    if not is_ds:
                        off = DS_OFF[mi] + (s - 128 * mi)
                        diff = work_pool.tile([128, 512], fp16, tag="tdiff")
                        nc.vector.scalar_tensor_tensor(
                            out=diff[:, :w],
                            in0=dtarget,
                            scalar=0.0,
                            in1=ds[:, off : off + w],
                            op0=Alu.bypass,
                            op1=Alu.subtract,
                        )
                        junk = work_pool.tile([128, 512], fp16, tag="tjunk")
                        nc.scalar.activation(
                            out=junk[:, :w],
                            in_=diff[:, :w],
                            func=Act.Square,
                            accum_out=sent_slots[:, si : si + 1],
                        )
                        si += 1
            # diag: rank-1 into a FRESH bank (start=True), combine with the
            # evacuated raw gram on DVE (avoids has_written loss from later
            # quarter-group starts), then sqrt from SBUF per quarter
            for h in range(2):
                r1ps = dg_pool.tile([128, 512], f32, tag="dps")
                nc.tensor.matmul(
                    r1ps[:],
                    ones_neg[:],
                    sqrow_f[0:1, 512 * h : 512 * (h + 1)],
                    start=True,
                    stop=True,
                )
                mdiag = work_pool.tile([128, 512], f32, tag="mdiag")
                nc.vector.scalar_tensor_tensor(
                    out=mdiag[:],
                    in0=raws[h][:],
                    scalar=0.0,
                    in1=r1ps[:],
                    op0=Alu.bypass,
                    op1=Alu.add,
                )
                for q in range(4):
                    mi = 4 * h + q
                    off = DS_OFF[mi]
                    if is_ds:
                        dtarget = ds[:, off : off + 128]
                    else:
                        ddiag = work_pool.tile([128, 128], fp16, tag="ddiag")
                        dtarget = ddiag[:]
                    nc.scalar.activation(
                        out=dtarget,
                        in_=mdiag[:, 128 * q : 128 * (q + 1)],
                        func=Act.Sqrt,
                        scale=-2.0,
                        bias=sbias[:, mi : mi + 1],
                    )
                    if not is_ds:
                        diffd = work_pool.tile([128, 128], fp16, tag="tdiffd")
                        nc.vector.scalar_tensor_tensor(
                            out=diffd[:],
                            in0=dtarget,
                            scalar=0.0,
                            in1=ds[:, off : off + 128],
                            op0=Alu.bypass,
                            op1=Alu.subtract,
                        )
                        junk2 = work_pool.tile([128, 128], fp16, tag="tjunk2")
                        nc.vector.scalar_tensor_tensor(
                            out=junk2[:],
                            in0=diffd[:],
                            scalar=0.0,
                            in1=diffd[:],
                            op0=Alu.bypass,
                            op1=Alu.mult,
                            accum_out=accd_slots[:, di : di + 1],
                        )
                        di += 1

        def secret_batch(l):
            """8 packed groups (32 b's): load l of 4."""
            xsn = xsn_pool.tile([128, 8, D], fp16, tag="xsn")
            for c0 in (0, 4):
                nc.gpsimd.dma_start(
                    xsn[:, c0 : c0 + 4, :],
                    xsp_ap[8 * l + c0 : 8 * l + c0 + 4].rearrange("g p d -> p g d"),
                )
            psbs = []
            for half in range(2):  # supergroup = 4 groups -> 1 gram bank
                psb = mm_pool.tile([128, 512], f32, tag="ps_mm")
                psbs.append(psb)
                for q in range(4):
                    gg = 4 * half + q
                    tp = tp_pool.tile([128, 8, 128], fp16, tag="tp")
                    for k in range(8):
                        nc.tensor.transpose(
                            tp[:, k, :],
                            xsn[:, gg, 128 * k : 128 * (k + 1)],
                            ident_b[:],
                        )
                    tg = tg_pool.tile([128, 8, 128], fp16, tag="tg")
                    nc.vector.tensor_copy(tg[:], tp[:])
                    for k in range(8):
                        nc.tensor.matmul(
                            psb[:, 128 * q : 128 * (q + 1)],
                            tg[:, k, :],
                            tg[:, k, :],
                            start=(k == 0),
                            stop=False,
                        )
            sq2s, sbias_s, sqsrow, raws_s = sq_chain(psbs, sqs_pool, 2, "s")
            sqsrow_f = sqsrow[:].rearrange("o t p -> o (t p)")
            for half in range(2):
                r1ps = mm_pool.tile([128, 512], f32, tag="ps_mm")
                nc.tensor.matmul(
                    r1ps[:],
                    ones_neg[:],
                    sqsrow_f[0:1, 512 * half : 512 * (half + 1)],
                    start=True,
                    stop=True,
                )
                msec = work_pool.tile([128, 512], f32, tag="msec")
                nc.vector.scalar_tensor_tensor(
                    out=msec[:],
                    in0=raws_s[half][:],
                    scalar=0.0,
                    in1=r1ps[:],
                    op0=Alu.bypass,
                    op1=Alu.add,
                )
                for q in range(4):
                    gg = 4 * half + q
                    g = 8 * l + gg
                    dsec = work_pool.tile([128, 128], fp16, tag="dsec")
                    nc.scalar.activation(
                        out=dsec[:],
                        in_=msec[:, 128 * q : 128 * (q + 1)],
                        func=Act.Sqrt,
                        scale=-2.0,
                        bias=sbias_s[:, gg : gg + 1],
                    )
                    hin = work_pool.tile([128, 128], fp16, tag="shin")
                    nc.scalar.activation(
                        out=hin[:],
                        in_=dsec[:],
                        func=Act.Relu,
                        scale=-1.0,
                        bias=float(MARGIN),
                    )
                    junk3 = work_pool.tile([128, 128], fp16, tag="sjunk3")
                    nc.vector.scalar_tensor_tensor(
                        out=junk3[:],
                        in0=hin[:],
                        scalar=0.0,
                        in1=pmask[:],
                        op0=Alu.bypass,
                        op1=Alu.mult,
                        accum_out=sec_slots[:, g : g + 1],
                    )

        # ---------------- interleaved schedule ----------------
        enc_src = enc_ap.rearrange("(t p) d -> p t d", p=128)
        xnat_enc = load_matrix(enc_src, 4)
        process_matrix(xnat_enc, True, 0, 0)
        for i in range(SECPC):
            xnat_i = load_matrix(xs_ap[i].rearrange("(t p) d -> p t d", p=128), 2)
            process_matrix(xnat_i, False, i * N_SEG, i * 8)
            secret_batch(i)

        # ---------------- final reduction + output ----------------
        with tc.tile_pool(name="outp", bufs=1) as opool:
            o_sent = opool.tile([128, 2], f32, tag="o_sent_sb")
            nc.vector.tensor_reduce(
                out=o_sent[:, 0:1], in_=sent_slots[:], axis=AxX, op=Alu.add
            )
            nc.vector.tensor_reduce(
                out=o_sent[:, 1:2], in_=accd_slots[:], axis=AxX, op=Alu.add
            )
            nc.sync.dma_start(o_sent_ap[:], o_sent[:])
            o_sec = opool.tile([128, 1], f32, tag="o_sec_sb")
            nc.vector.tensor_reduce(
                out=o_sec[:], in_=sec_slots[:], axis=AxX, op=Alu.add
            )
            nc.sync.dma_start(o_sec_ap[:], o_sec[:])


_NC_CACHE = None


def _get_nc():
    global _NC_CACHE
    if _NC_CACHE is None:
        _NC_CACHE = _build()
    return _NC_CACHE


def _host_inputs():
    ident_b = np.eye(128, dtype=np.float16)
    bb = np.arange(128) // 32
    ii = np.arange(128) % 32
    pm = ((bb[:, None] == bb[None, :]) & (ii[:, None] < ii[None, :])).astype(
        np.float16
    )
    return ident_b, pm


def run_on_device(outputs, encode_sentences, trace=False, **kw):
    nc = _get_nc()
    ident_b, pm = _host_inputs()
    in_maps = []
    for c in range(NCORES):
        xsl = outputs[:, BSH * c : BSH * (c + 1), :]  # [32, 128, D]
        xsp = np.ascontiguousarray(xsl.transpose(1, 0, 2)).reshape(NG, 4 * N, D)
        in_maps.append(
            {
                "xs": np.ascontiguousarray(outputs[SECPC * c : SECPC * (c + 1)]),
                "xsecp": xsp,
                "enc": np.ascontiguousarray(encode_sentences),
                "identb": ident_b,
                "pmask": pm,
            }
        )
    return run_bass_kernel_spmd(nc, in_maps, list(range(NCORES)), trace=trace, **kw)


def _finish(results):
    sent_nondiag = 0.0
    diag = 0.0
    sec = 0.0
    for c in range(NCORES):
        r = results[c]
        sent_nondiag += r["o_sent"][:, 0].sum(dtype=np.float64)
        diag += r["o_sent"][:, 1].sum(dtype=np.float64)
        sec += r["o_sec"].sum(dtype=np.float64)
    # full = 2 * (strict upper-block region) + diag blocks
    total_sent = 2.0 * sent_nondiag + diag
    sentence_loss = total_sent / (N * B * B)
    secret_loss = (sec / B) / (N * (N - 1) / 2.0)
    loss = ALPHA * sentence_loss + (1.0 - ALPHA) * secret_loss
    return (
        np.float32(loss),
        np.float32(sentence_loss),
        np.float32(secret_loss),
    )


def kernel(outputs, encode_sentences):
    res = run_on_device(outputs, encode_sentences)
    return _finish(res.results)
